# revision 3
# baseline (speedup 1.0000x reference)
"""BatchAllTripletLoss v4: (slot, rank) windowed layout, 8-core SPMD.

Each core's 128 anchor slots are a contiguous 128-point window of the
label-sorted embedding (the host rolls the sorted array per core so the
window sits at columns 0:128 of that core's ET).  Anchors whose class
needs more positive-ranks than T are covered by several overlapping
windows (greedy interval multicover), so every core runs the same
T = max-ranks-per-slot tiles (T=12 here, provably minimal: 8*128 slot
instances must cover sum_p ceil((m_p-1)/T) requirements).

Phase A: the pd row block [128 slots x 512 pts] comes from six fp32r
matmuls into one PSUM bank, then one ACT Sqrt to f32.  pd is recentered
by sqrt(2D) and cast to f16 (one f16 ulp ~0.002 << margin 0.1; raw
distances concentrate at ~22.6 where even bf16/f16 ulps would swamp the
margin).  bias = pdc + host K-mask (same-class columns pushed out of
relu range).  ap' values pdc[slot, positive_t] come from one gpsimd
indirect_copy (indices are shared per 16-partition group, so it gathers
a 16x16 block per tile) followed by a diagonal-extract multiply with a
host 0/1 mask and a tensor_reduce.

Phase B sweeps the SAME f16 bias tile for every rank (no replication
matmul, no PSUM).  On HW every reduction (accum_out) runs ~1x and
costs ~600-800ns at N=512 regardless of DVE perf mode, so reductions
are batched: cheap 4x-mode DVE passes write R_t = bias - ap_t into a
stacked [128, T, 512] tile, then ONE ACT relu+accum over all T*512
columns yields the grand loss sum, one batched ACT Sign pass covers
T//3 tiles' counts (per-partition sign-sum = 2C - n*512, undone on
host), and the remaining counts are per-tile DVE is_lt+accum ops.
KMASK/ABIG are sized so |bias - ap| stays inside f16 range.

The timing builds wrap each phase in a hardware For_i whose body holds
8 copies of the phase: this amortizes the loop's all-engine barrier
(~1.7us) and lets copies pipeline through 4-way-buffered tiles.
"""
import os
import sys

for _p in ("/opt/trn_rl_repo",):
    if os.path.isdir(_p) and _p not in sys.path:
        sys.path.insert(0, _p)

import numpy as np

import concourse.bacc as bacc
import concourse.tile as tile
from concourse import mybir
from concourse import bass_utils

N = 512
D = 256
N_CORES = 8
NCLS = 32
MARGIN = 0.1
EPS = 1e-16
D2_EPS = 0.05               # dominates fp32r rounding noise at d2~0
KMASK = 1024.0              # added to bias at same-class columns
ABIG = -46000.0             # apcol value for invalid (slot, rank)
                            # (|ABIG| + KMASK + |pdc| must stay < f16 max)
HALF = 0.70710678118654752  # sqrt(1/2): Square(x*HALF) = x^2/2
CENTER = 22.62741699796952    # sqrt(2D): distances concentrate here; pd is
                              # recentered before the f16 cast so one f16 ulp
                              # (~0.002) stays far below the 0.1 margin

F32 = mybir.dt.float32
F32R = mybir.dt.float32r
BF16 = mybir.dt.bfloat16
U16 = mybir.dt.uint16
F16 = mybir.dt.float16
BF16_NP = mybir.dt.np(mybir.dt.bfloat16)
F16_NP = mybir.dt.np(mybir.dt.float16)
AF = mybir.ActivationFunctionType
OP = mybir.AluOpType
AX = mybir.AxisListType

_PROGRAM_CACHE = {}


def n_sign(T):
    """Tiles whose count runs on ACT via a batched Sign pass (the last
    n_sign tiles); the rest count on DVE via is_lt+accum."""
    return T // 3


def blob_layout(T):
    T2 = (T + 1) // 2
    c_apM = T2
    c_apC = T2 + T
    c_E16 = T2 + 2 * T
    c_eps = c_E16 + 8 * T
    B = c_eps + 1
    return T2, c_apM, c_apC, c_E16, c_eps, B


def build_program(T, n_rep=1, loop=None, unroll=1):
    """loop=None: single-shot. loop="B": For_i around phase B.
    loop="A": For_i around input DMAs + phase A.  The loop body holds
    `unroll` copies of the phase (amortizes the For_i all-engine
    barrier and lets consecutive copies pipeline); n_rep counts phase
    executions and must be divisible by unroll."""
    NS = n_sign(T)
    T2, c_apM, c_apC, c_E16, c_eps, B = blob_layout(T)
    assert n_rep % unroll == 0
    nc = bacc.Bacc(trn_type="TRN2")

    et_d = nc.dram_tensor("ET", [128, 2, N], F32R, kind="ExternalInput")
    msk_d = nc.dram_tensor("mskb", [128, N], F16, kind="ExternalInput")
    blob_d = nc.dram_tensor("blob", [128, B], F32, kind="ExternalInput")
    out_d = nc.dram_tensor("out", [128, 2 * T], F32, kind="ExternalOutput")

    with tile.TileContext(nc) as tc:
        with tc.tile_pool(name="persist", bufs=1) as persist, \
             tc.tile_pool(name="pa", bufs=4) as pa, \
             tc.tile_pool(name="psA", bufs=4, space="PSUM") as psA, \
             tc.tile_pool(name="rs", bufs=2) as rs, \
             tc.tile_pool(name="sc", bufs=4) as sc:

            neg_sb = persist.tile([128, N], F32)
            out_sb = persist.tile([128, 2 * T], F32)
            dum_sb = persist.tile([1, 1], F32)
            one_sb = persist.tile([1, 1], F32)

            def setup():
                nc.vector.memset(neg_sb[:], -1.0)
                nc.vector.memset(out_sb[:], 0.0)
                nc.vector.memset(one_sb[:], 1.0)
                # pin the sqrt_and_others ACT table once, off-critical-path
                nc.scalar.activation(dum_sb[:], one_sb[:], AF.Sqrt)

            def phase_a():
                """input DMAs + pd/bias/apcol prep; returns (bias, apcol).
                All tiles come from the double-buffered `pa` pool so
                consecutive copies can overlap."""
                et_sb = pa.tile([128, 2, N], F32R, tag="et")
                msk_sb = pa.tile([128, N], F16, tag="msk")
                blob_sb = pa.tile([128, B], F32, tag="blob")
                sq2_sb = pa.tile([128, 2, N], F32R, tag="sq2")
                pd_sb = pa.tile([128, N], F32, tag="pd")
                pdc_sb = pa.tile([128, N], F16, tag="pdc")
                bias_sb = pa.tile([128, N], F16, tag="bias")
                diag_sb = pa.tile([128, T, 16], F16, tag="diag")
                prod_sb = pa.tile([128, T, 16], F16, tag="prod")
                apraw_sb = pa.tile([128, T], F32, tag="apraw")
                apcol_sb = pa.tile([128, T], F32, tag="apcol")

                idx_v = blob_sb[:, 0:T2].bitcast(U16)
                apM_v = blob_sb[:, c_apM:c_apM + T]
                apC_v = blob_sb[:, c_apC:c_apC + T]
                e16_v = blob_sb[:, c_E16:c_E16 + 8 * T].bitcast(F16)
                eps_v = blob_sb[:, c_eps:c_eps + 1]

                # ET split per half so the h=0 gram matmul starts after
                # half the bytes; small DMAs ride the ACT HWDGE ring to
                # stay off the big transfer's SP ring
                nc.sync.dma_start(et_sb[:, 0, :], et_d.ap()[:, 0, :])
                nc.sync.dma_start(et_sb[:, 1, :], et_d.ap()[:, 1, :])
                nc.scalar.dma_start(msk_sb[:], msk_d.ap()[:])
                nc.scalar.dma_start(blob_sb[:], blob_d.ap()[:])

                d2 = psA.tile([128, N], F32, tag="d2")
                nc.tensor.matmul(d2[:], lhsT=et_sb[:, 0, 0:128],
                                 rhs=et_sb[:, 0, :],
                                 start=True, stop=False)
                for h in range(2):
                    nc.scalar.activation(sq2_sb[:, h, :], et_sb[:, h, :],
                                         AF.Square, scale=HALF)
                nc.tensor.matmul(d2[:], lhsT=et_sb[:, 1, 0:128],
                                 rhs=et_sb[:, 1, :],
                                 start=False, stop=False)
                for h in range(2):
                    nc.tensor.matmul(d2[:], lhsT=sq2_sb[:, h, 0:128],
                                     rhs=neg_sb.bitcast(F32R)[:],
                                     start=False, stop=False)
                for h in range(2):
                    nc.tensor.matmul(d2[:], lhsT=neg_sb.bitcast(F32R)[:, 0:128],
                                     rhs=sq2_sb[:, h, :],
                                     start=False, stop=(h == 1))
                # pd = sqrt(-2*psum + eps)
                nc.scalar.activation(pd_sb[:], d2[:], AF.Sqrt,
                                     bias=eps_v, scale=-2.0)
                # recenter so f16 keeps ~0.002 resolution near the margin
                nc.vector.tensor_scalar(pdc_sb[:], pd_sb[:], CENTER, 0.0,
                                        op0=OP.subtract, op1=OP.add)
                # ap' gather: diag[p, 16t+q] = pdc[p, idxJ[16g+q, t]]
                nc.gpsimd.indirect_copy(
                    diag_sb.rearrange("p a b -> p (a b)"),
                    pdc_sb[:], idx_v[:, 0:T], True)
                # bias rows: pd + KMASK at same-class columns
                nc.vector.tensor_tensor(bias_sb[:], pdc_sb[:], msk_sb[:],
                                        op=OP.add)
                # diagonal extract: apraw[p,t] = sum_q diag[p,16t+q]*E16[p,q]
                nc.vector.tensor_tensor(
                    prod_sb.rearrange("p a b -> p (a b)"),
                    diag_sb.rearrange("p a b -> p (a b)"),
                    e16_v, op=OP.mult)
                nc.vector.tensor_reduce(apraw_sb[:], prod_sb[:],
                                        axis=AX.X, op=OP.add)
                # apcol = apraw*apM + apC  (valid: ap'+margin, else -BIG)
                nc.vector.tensor_tensor(apraw_sb[:], apraw_sb[:], apM_v,
                                        op=OP.mult)
                nc.vector.tensor_tensor(apcol_sb[:], apraw_sb[:], apC_v,
                                        op=OP.add)
                return bias_sb, apcol_sb

            def phase_b(bias_sb, apcol_sb):
                # R_t = bias - ap_t via cheap 4x-mode passes, stacked
                Rstk = rs.tile([128, T, N], F16, tag="Rstk")
                for t in range(T):
                    nc.vector.tensor_scalar(
                        Rstk[:, t, :], bias_sb[:], apcol_sb[:, t:t + 1],
                        0.0, op0=OP.subtract, op1=OP.add)
                # grand loss sum: ONE ACT relu+accum over all T tiles
                # (amortizes the ~370ns fixed cost per reduction op)
                J = rs.tile([128, T, N], F16, tag="J")
                nc.scalar.activation(
                    J.rearrange("p a b -> p (a b)"),
                    Rstk.rearrange("p a b -> p (a b)"),
                    AF.Relu, scale=-1.0,
                    accum_out=out_sb[:, 0:1])
                # counts: last NS tiles batched on ACT via Sign
                # (sign-sum = 2C - NS*512), the rest on DVE is_lt+accum
                G2 = rs.tile([128, T, N], F16, tag="J")
                nc.scalar.activation(
                    G2[:, 0:NS, :].rearrange("p a b -> p (a b)"),
                    Rstk[:, T - NS:T, :].rearrange("p a b -> p (a b)"),
                    AF.Sign, scale=-1.0,
                    accum_out=out_sb[:, T:T + 1])
                for t in range(T - NS):
                    G = sc.tile([128, N], F16, tag="G")
                    nc.vector.tensor_scalar(
                        G[:], Rstk[:, t, :], 0.0, None,
                        op0=OP.is_lt, op1=OP.add,
                        accum_out=out_sb[:, T + 1 + t:T + 2 + t])

            setup()
            if loop is None:
                ba = phase_a()
                phase_b(*ba)
            elif loop == "B":
                ba = phase_a()
                with tc.For_i(0, n_rep // unroll, 1):
                    for _ in range(unroll):
                        phase_b(*ba)
            elif loop == "A":
                with tc.For_i(0, n_rep // unroll, 1):
                    for _ in range(unroll):
                        phase_a()
            else:
                raise ValueError(loop)

            nc.sync.dma_start(out_d.ap()[:], out_sb[:])

    nc.compile()
    return nc


def get_program(T, n_rep=1, loop=None):
    unroll = 8 if (loop is not None and n_rep % 8 == 0) else 1
    key = (T, n_rep, loop, unroll)
    if key not in _PROGRAM_CACHE:
        _PROGRAM_CACHE[key] = build_program(T, n_rep, loop, unroll)
    return _PROGRAM_CACHE[key]


def host_layout(labels):
    """Label-sort; pick minimal T such that 8 windows of 128 contiguous
    sorted positions can cover every anchor position P with multiplicity
    ceil((m_P - 1)/T) (each covering handles up to T positive-ranks).
    Classic greedy interval multicover: repeatedly place a window at the
    first position with unmet requirement.  Returns (perm, counts,
    starts, T, windows); windows = list of (window_start,
    {abs_position: (r0, r1)})."""
    lab = np.asarray(labels).astype(np.int64)
    counts = np.bincount(lab, minlength=NCLS)
    perm = np.argsort(lab, kind="stable")
    starts = np.zeros(NCLS + 1, dtype=np.int64)
    starts[1:] = np.cumsum(counts)
    lab_s = lab[perm]
    need = np.maximum(counts[lab_s] - 1, 0)      # ranks needed per position

    wins = []
    for T in range(1, 64):
        req = -(-need // T)                      # ceil
        covered = np.zeros(N, dtype=np.int64)
        wins = []
        ok = True
        while True:
            unmet = np.nonzero(covered < req)[0]
            if len(unmet) == 0:
                break
            p = int(unmet[0])
            wins.append(p)
            covered[p:p + 128] += 1
            if len(wins) > N_CORES:
                ok = False
                break
        if ok:
            break

    windows = []
    taken = np.zeros(N, dtype=np.int64)
    for ws in wins:
        asg = {}
        for p in range(ws, min(ws + 128, N)):
            rem = int(need[p] - taken[p])
            if rem > 0:
                take = min(T, rem)
                asg[p] = (int(taken[p]), int(taken[p]) + take)
                taken[p] += take
        windows.append((ws, asg))
    while len(windows) < N_CORES:
        windows.append((0, {}))
    return perm, counts, starts, T, windows


def make_in_maps(embeddings, labels):
    emb = np.ascontiguousarray(np.asarray(embeddings, dtype=np.float32))
    assert emb.shape == (N, D)
    perm, counts, starts, T, windows = host_layout(labels)
    emb_s = emb[perm]
    lab_s = np.asarray(labels).astype(np.int64)[perm]
    T2, c_apM, c_apC, c_E16, c_eps, B = blob_layout(T)

    e16t = np.zeros((128, 16 * T), dtype=np.float32)
    p16 = np.arange(128) % 16
    for t in range(T):
        e16t[np.arange(128), 16 * t + p16] = 1.0

    in_maps = []
    for q in range(N_CORES):
        ws, asg = windows[q]
        cols = (ws + np.arange(N)) % N
        emb_r = emb_s[cols]                       # [512, 256]
        lab_r = lab_s[cols]
        ET = np.ascontiguousarray(
            emb_r.T.reshape(2, 128, N).transpose(1, 0, 2))   # [128,2,512]
        mskb = (KMASK * (lab_r[:128, None] == lab_r[None, :])).astype(F16_NP)

        idxJ = np.zeros((128, T), dtype=np.uint16)
        apM = np.zeros((128, T), dtype=np.float32)
        apC = np.full((128, T), ABIG, dtype=np.float32)
        for p_abs, (r0, r1) in asg.items():
            slot = int(p_abs - ws)
            c = lab_s[p_abs]
            members = np.arange(starts[c], starts[c + 1])
            others = members[members != p_abs]
            for t in range(r1 - r0):
                j_abs = others[r0 + t]
                idxJ[slot, t] = (j_abs - ws) % N
                apM[slot, t] = 1.0
                apC[slot, t] = MARGIN

        blob = np.zeros((128, B), dtype=np.float32)
        blob[:, 0:T2].view(np.uint16)[:, 0:T] = idxJ
        blob[:, c_apM:c_apM + T] = apM
        blob[:, c_apC:c_apC + T] = apC
        blob[:, c_E16:c_E16 + 8 * T].view(F16_NP)[:, :] = e16t.astype(F16_NP)
        blob[:, c_eps] = D2_EPS

        in_maps.append({"ET": ET, "mskb": mskb, "blob": blob})
    return in_maps, T


def reduce_outputs(results, T):
    NS = n_sign(T)
    loss_sum = 0.0
    hard_sum = 0.0
    for r in results:
        o = r["out"].astype(np.float64)
        loss_sum += o[:, 0].sum()
        # sign-sum per partition over NS tiles: 2C - NS*512 -> C
        hard_sum += ((o[:, T] + NS * N) / 2.0).sum()
        for t in range(T - NS):
            hard_sum += o[:, T + 1 + t].sum()
    num_hard = np.float32(hard_sum)
    loss = np.float32(np.float32(loss_sum) / (num_hard + np.float32(EPS)))
    return loss, num_hard


def kernel(embeddings, labels):
    in_maps, T = make_in_maps(embeddings, labels)
    nc = get_program(T)
    res = bass_utils.run_bass_kernel_spmd(
        nc, in_maps, core_ids=list(range(N_CORES)))
    return reduce_outputs(res.results, T)


# revision 5
# speedup vs baseline: 1.1472x; 1.1472x over previous
"""BatchAllTripletLoss v4: (slot, rank) windowed layout, 8-core SPMD.

Each core's 128 anchor slots are a contiguous 128-point window of the
label-sorted embedding (the host rolls the sorted array per core so the
window sits at columns 0:128 of that core's ET).  Anchors whose class
needs more positive-ranks than T are covered by several overlapping
windows (greedy interval multicover), so every core runs the same
T = max-ranks-per-slot tiles (T=12 here, provably minimal: 8*128 slot
instances must cover sum_p ceil((m_p-1)/T) requirements).

Phase A: the pd row block [128 slots x 512 pts] comes from six fp32r
matmuls into one PSUM bank, then one ACT Sqrt to f32.  pd is recentered
by sqrt(2D) and cast to f16 (one f16 ulp ~0.002 << margin 0.1; raw
distances concentrate at ~22.6 where even bf16/f16 ulps would swamp the
margin).  bias = pdc + host K-mask (same-class columns pushed out of
relu range).  ap' values pdc[slot, positive_t] come from one gpsimd
indirect_copy (indices are shared per 16-partition group, so it gathers
a 16x16 block per tile) followed by a diagonal-extract multiply with a
host 0/1 mask and a tensor_reduce.

Phase B sweeps the SAME f16 bias tile for every rank (no replication
matmul, no PSUM).  On HW every reduction (accum_out) runs ~1x and
costs ~600-800ns at N=512 regardless of DVE perf mode, so reductions
are batched: cheap 4x-mode DVE passes write R_t = bias - ap_t into a
stacked [128, T, 512] tile, then ONE ACT relu+accum over all T*512
columns yields the grand loss sum, one batched ACT Sign pass covers
T//3 tiles' counts (per-partition sign-sum = 2C - n*512, undone on
host), and the remaining counts are per-tile DVE is_lt+accum ops.
KMASK/ABIG are sized so |bias - ap| stays inside f16 range.

The timing builds wrap each phase in a hardware For_i whose body holds
8 copies of the phase: this amortizes the loop's all-engine barrier
(~1.7us) and lets copies pipeline through 4-way-buffered tiles.
"""
import os
import sys

for _p in ("/opt/trn_rl_repo",):
    if os.path.isdir(_p) and _p not in sys.path:
        sys.path.insert(0, _p)

import numpy as np

import concourse.bacc as bacc
import concourse.tile as tile
from concourse import mybir
from concourse import bass_utils

N = 512
D = 256
N_CORES = 8
NCLS = 32
MARGIN = 0.1
EPS = 1e-16
D2_EPS = 0.05               # dominates fp32r rounding noise at d2~0
KMASK = 1024.0              # added to bias at same-class columns
ABIG = -46000.0             # apcol value for invalid (slot, rank)
                            # (|ABIG| + KMASK + |pdc| must stay < f16 max)
HALF = 0.70710678118654752  # sqrt(1/2): Square(x*HALF) = x^2/2
CENTER = 22.62741699796952    # sqrt(2D): distances concentrate here; pd is
                              # recentered before the f16 cast so one f16 ulp
                              # (~0.002) stays far below the 0.1 margin

F32 = mybir.dt.float32
F32R = mybir.dt.float32r
BF16 = mybir.dt.bfloat16
U16 = mybir.dt.uint16
F16 = mybir.dt.float16
BF16_NP = mybir.dt.np(mybir.dt.bfloat16)
F16_NP = mybir.dt.np(mybir.dt.float16)
AF = mybir.ActivationFunctionType
OP = mybir.AluOpType
AX = mybir.AxisListType

_PROGRAM_CACHE = {}


def n_sign(T):
    """Tiles whose count runs on ACT via a batched Sign pass (the last
    n_sign tiles); the rest count on DVE via is_lt+accum."""
    return T // 3


def blob_layout(T):
    T2 = (T + 1) // 2
    c_apM = T2
    c_apC = T2 + T
    c_E16 = T2 + 2 * T
    c_eps = c_E16 + 8 * T
    c_msk = c_eps + 1
    B = c_msk + N // 2          # K-mask rides in the blob as f16
    return T2, c_apM, c_apC, c_E16, c_eps, c_msk, B


def build_program(T, n_rep=1, loop=None, unroll=1):
    """loop=None: single-shot. loop="B": For_i around phase B.
    loop="A": For_i around input DMAs + phase A.  The loop body holds
    `unroll` copies of the phase (amortizes the For_i all-engine
    barrier and lets consecutive copies pipeline); n_rep counts phase
    executions and must be divisible by unroll."""
    NS = n_sign(T)
    T2, c_apM, c_apC, c_E16, c_eps, c_msk, B = blob_layout(T)
    assert n_rep % unroll == 0
    nc = bacc.Bacc(trn_type="TRN2")

    et_d = nc.dram_tensor("ET", [128, 2, N], F32R, kind="ExternalInput")
    blob_d = nc.dram_tensor("blob", [128, B], F32, kind="ExternalInput")
    out_d = nc.dram_tensor("out", [128, 2 * T], F32, kind="ExternalOutput")

    with tile.TileContext(nc) as tc:
        with tc.tile_pool(name="persist", bufs=1) as persist, \
             tc.tile_pool(name="pa", bufs=4) as pa, \
             tc.tile_pool(name="psA", bufs=4, space="PSUM") as psA, \
             tc.tile_pool(name="rs", bufs=3) as rs, \
             tc.tile_pool(name="sc", bufs=4) as sc:

            neg_sb = persist.tile([128, N], F32)
            out_sb = persist.tile([128, 2 * T], F32)
            dum_sb = persist.tile([1, 1], F32)
            one_sb = persist.tile([1, 1], F32)

            def setup():
                nc.vector.memset(neg_sb[:], -1.0)
                nc.vector.memset(out_sb[:], 0.0)
                nc.vector.memset(one_sb[:], 1.0)
                # pin the sqrt_and_others ACT table once, off-critical-path
                nc.scalar.activation(dum_sb[:], one_sb[:], AF.Sqrt)

            def phase_a():
                """input DMAs + pd/bias/apcol prep; returns (bias, apcol).
                All tiles come from the double-buffered `pa` pool so
                consecutive copies can overlap."""
                et_sb = pa.tile([128, 2, N], F32R, tag="et")
                blob_sb = pa.tile([128, B], F32, tag="blob")
                sq2_sb = pa.tile([128, 2, N], F32R, tag="sq2")
                pd_sb = pa.tile([128, N], F32, tag="pd")
                pdc_sb = pa.tile([128, N], F16, tag="pdc")
                bias_sb = pa.tile([128, N], F16, tag="bias")
                diag_sb = pa.tile([128, T, 16], F16, tag="diag")
                prod_sb = pa.tile([128, T, 16], F16, tag="prod")
                apraw_sb = pa.tile([128, T], F32, tag="apraw")
                apcol_sb = pa.tile([128, T], F32, tag="apcol")

                idx_v = blob_sb[:, 0:T2].bitcast(U16)
                apM_v = blob_sb[:, c_apM:c_apM + T]
                apC_v = blob_sb[:, c_apC:c_apC + T]
                e16_v = blob_sb[:, c_E16:c_E16 + 8 * T].bitcast(F16)
                eps_v = blob_sb[:, c_eps:c_eps + 1]
                msk_v = blob_sb[:, c_msk:c_msk + N // 2].bitcast(F16)

                # ET split per half so the h=0 gram matmul starts after
                # half the bytes; small DMAs ride the ACT HWDGE ring to
                # stay off the big transfer's SP ring
                nc.sync.dma_start(et_sb[:, 0, :], et_d.ap()[:, 0, :])
                nc.sync.dma_start(et_sb[:, 1, :], et_d.ap()[:, 1, :])
                nc.scalar.dma_start(blob_sb[:], blob_d.ap()[:])

                d2 = psA.tile([128, N], F32, tag="d2")
                nc.tensor.matmul(d2[:], lhsT=et_sb[:, 0, 0:128],
                                 rhs=et_sb[:, 0, :],
                                 start=True, stop=False)
                for h in range(2):
                    nc.scalar.activation(sq2_sb[:, h, :], et_sb[:, h, :],
                                         AF.Square, scale=HALF)
                nc.tensor.matmul(d2[:], lhsT=et_sb[:, 1, 0:128],
                                 rhs=et_sb[:, 1, :],
                                 start=False, stop=False)
                for h in range(2):
                    nc.tensor.matmul(d2[:], lhsT=sq2_sb[:, h, 0:128],
                                     rhs=neg_sb.bitcast(F32R)[:],
                                     start=False, stop=False)
                for h in range(2):
                    nc.tensor.matmul(d2[:], lhsT=neg_sb.bitcast(F32R)[:, 0:128],
                                     rhs=sq2_sb[:, h, :],
                                     start=False, stop=(h == 1))
                # pd = sqrt(-2*psum + eps)
                nc.scalar.activation(pd_sb[:], d2[:], AF.Sqrt,
                                     bias=eps_v, scale=-2.0)
                # recenter so f16 keeps ~0.002 resolution near the margin
                nc.vector.tensor_scalar(pdc_sb[:], pd_sb[:], CENTER, 0.0,
                                        op0=OP.subtract, op1=OP.add)
                # ap' gather: diag[p, 16t+q] = pdc[p, idxJ[16g+q, t]]
                nc.gpsimd.indirect_copy(
                    diag_sb.rearrange("p a b -> p (a b)"),
                    pdc_sb[:], idx_v[:, 0:T], True)
                # bias rows: pd + KMASK at same-class columns
                nc.vector.tensor_tensor(bias_sb[:], pdc_sb[:], msk_v,
                                        op=OP.add)
                # diagonal extract: apraw[p,t] = sum_q diag[p,16t+q]*E16[p,q]
                nc.vector.tensor_tensor(
                    prod_sb.rearrange("p a b -> p (a b)"),
                    diag_sb.rearrange("p a b -> p (a b)"),
                    e16_v, op=OP.mult)
                nc.vector.tensor_reduce(apraw_sb[:], prod_sb[:],
                                        axis=AX.X, op=OP.add)
                # apcol = apraw*apM + apC  (valid: ap'+margin, else -BIG)
                nc.vector.tensor_tensor(apraw_sb[:], apraw_sb[:], apM_v,
                                        op=OP.mult)
                nc.vector.tensor_tensor(apcol_sb[:], apraw_sb[:], apC_v,
                                        op=OP.add)
                return bias_sb, apcol_sb

            def phase_b(bias_sb, apcol_sb):
                # R_t = bias - ap_t via cheap 4x-mode passes, stacked
                Rstk = rs.tile([128, T, N], F16, tag="Rstk")
                for t in range(T):
                    nc.vector.tensor_scalar(
                        Rstk[:, t, :], bias_sb[:], apcol_sb[:, t:t + 1],
                        0.0, op0=OP.subtract, op1=OP.add)
                # grand loss sum: ONE ACT relu+accum over all T tiles
                # (amortizes the ~370ns fixed cost per reduction op)
                J = rs.tile([128, T, N], F16, tag="J")
                nc.scalar.activation(
                    J.rearrange("p a b -> p (a b)"),
                    Rstk.rearrange("p a b -> p (a b)"),
                    AF.Relu, scale=-1.0,
                    accum_out=out_sb[:, 0:1])
                # counts: last NS tiles batched on ACT via Sign
                # (sign-sum = 2C - NS*512), the rest on DVE is_lt+accum
                G2 = rs.tile([128, T, N], F16, tag="J")
                nc.scalar.activation(
                    G2[:, 0:NS, :].rearrange("p a b -> p (a b)"),
                    Rstk[:, T - NS:T, :].rearrange("p a b -> p (a b)"),
                    AF.Sign, scale=-1.0,
                    accum_out=out_sb[:, T:T + 1])
                for t in range(T - NS):
                    G = sc.tile([128, N], F16, tag="G")
                    nc.vector.tensor_scalar(
                        G[:], Rstk[:, t, :], 0.0, None,
                        op0=OP.is_lt, op1=OP.add,
                        accum_out=out_sb[:, T + 1 + t:T + 2 + t])

            setup()
            if loop is None:
                ba = phase_a()
                phase_b(*ba)
            elif loop == "B":
                ba = phase_a()
                with tc.For_i(0, n_rep // unroll, 1, staggered_reset=True):
                    for _ in range(unroll):
                        phase_b(*ba)
            elif loop == "A":
                with tc.For_i(0, n_rep // unroll, 1, staggered_reset=True):
                    for _ in range(unroll):
                        phase_a()
            else:
                raise ValueError(loop)

            nc.sync.dma_start(out_d.ap()[:], out_sb[:])

    nc.compile()
    return nc


def get_program(T, n_rep=1, loop=None):
    unroll = 8 if (loop is not None and n_rep % 8 == 0) else 1
    key = (T, n_rep, loop, unroll)
    if key not in _PROGRAM_CACHE:
        _PROGRAM_CACHE[key] = build_program(T, n_rep, loop, unroll)
    return _PROGRAM_CACHE[key]


def host_layout(labels):
    """Label-sort; pick minimal T such that 8 windows of 128 contiguous
    sorted positions can cover every anchor position P with multiplicity
    ceil((m_P - 1)/T) (each covering handles up to T positive-ranks).
    Classic greedy interval multicover: repeatedly place a window at the
    first position with unmet requirement.  Returns (perm, counts,
    starts, T, windows); windows = list of (window_start,
    {abs_position: (r0, r1)})."""
    lab = np.asarray(labels).astype(np.int64)
    counts = np.bincount(lab, minlength=NCLS)
    perm = np.argsort(lab, kind="stable")
    starts = np.zeros(NCLS + 1, dtype=np.int64)
    starts[1:] = np.cumsum(counts)
    lab_s = lab[perm]
    need = np.maximum(counts[lab_s] - 1, 0)      # ranks needed per position

    wins = []
    for T in range(1, 64):
        req = -(-need // T)                      # ceil
        covered = np.zeros(N, dtype=np.int64)
        wins = []
        ok = True
        while True:
            unmet = np.nonzero(covered < req)[0]
            if len(unmet) == 0:
                break
            p = int(unmet[0])
            wins.append(p)
            covered[p:p + 128] += 1
            if len(wins) > N_CORES:
                ok = False
                break
        if ok:
            break

    windows = []
    taken = np.zeros(N, dtype=np.int64)
    for ws in wins:
        asg = {}
        for p in range(ws, min(ws + 128, N)):
            rem = int(need[p] - taken[p])
            if rem > 0:
                take = min(T, rem)
                asg[p] = (int(taken[p]), int(taken[p]) + take)
                taken[p] += take
        windows.append((ws, asg))
    while len(windows) < N_CORES:
        windows.append((0, {}))
    return perm, counts, starts, T, windows


def make_in_maps(embeddings, labels):
    emb = np.ascontiguousarray(np.asarray(embeddings, dtype=np.float32))
    assert emb.shape == (N, D)
    perm, counts, starts, T, windows = host_layout(labels)
    emb_s = emb[perm]
    lab_s = np.asarray(labels).astype(np.int64)[perm]
    T2, c_apM, c_apC, c_E16, c_eps, c_msk, B = blob_layout(T)

    e16t = np.zeros((128, 16 * T), dtype=np.float32)
    p16 = np.arange(128) % 16
    for t in range(T):
        e16t[np.arange(128), 16 * t + p16] = 1.0

    in_maps = []
    for q in range(N_CORES):
        ws, asg = windows[q]
        cols = (ws + np.arange(N)) % N
        emb_r = emb_s[cols]                       # [512, 256]
        lab_r = lab_s[cols]
        ET = np.ascontiguousarray(
            emb_r.T.reshape(2, 128, N).transpose(1, 0, 2))   # [128,2,512]
        mskb = (KMASK * (lab_r[:128, None] == lab_r[None, :])).astype(F16_NP)

        idxJ = np.zeros((128, T), dtype=np.uint16)
        apM = np.zeros((128, T), dtype=np.float32)
        apC = np.full((128, T), ABIG, dtype=np.float32)
        for p_abs, (r0, r1) in asg.items():
            slot = int(p_abs - ws)
            c = lab_s[p_abs]
            members = np.arange(starts[c], starts[c + 1])
            others = members[members != p_abs]
            for t in range(r1 - r0):
                j_abs = others[r0 + t]
                idxJ[slot, t] = (j_abs - ws) % N
                apM[slot, t] = 1.0
                apC[slot, t] = MARGIN

        blob = np.zeros((128, B), dtype=np.float32)
        blob[:, 0:T2].view(np.uint16)[:, 0:T] = idxJ
        blob[:, c_apM:c_apM + T] = apM
        blob[:, c_apC:c_apC + T] = apC
        blob[:, c_E16:c_E16 + 8 * T].view(F16_NP)[:, :] = e16t.astype(F16_NP)
        blob[:, c_eps] = D2_EPS
        blob[:, c_msk:c_msk + N // 2].view(F16_NP)[:, :] = mskb

        in_maps.append({"ET": ET, "blob": blob})
    return in_maps, T


def reduce_outputs(results, T):
    NS = n_sign(T)
    loss_sum = 0.0
    hard_sum = 0.0
    for r in results:
        o = r["out"].astype(np.float64)
        loss_sum += o[:, 0].sum()
        # sign-sum per partition over NS tiles: 2C - NS*512 -> C
        hard_sum += ((o[:, T] + NS * N) / 2.0).sum()
        for t in range(T - NS):
            hard_sum += o[:, T + 1 + t].sum()
    num_hard = np.float32(hard_sum)
    loss = np.float32(np.float32(loss_sum) / (num_hard + np.float32(EPS)))
    return loss, num_hard


def kernel(embeddings, labels):
    in_maps, T = make_in_maps(embeddings, labels)
    nc = get_program(T)
    res = bass_utils.run_bass_kernel_spmd(
        nc, in_maps, core_ids=list(range(N_CORES)))
    return reduce_outputs(res.results, T)
